# revision 13
# baseline (speedup 1.0000x reference)
"""MAMGCN submodule kernel for Trainium2, 8-core data-parallel over batch.

Problem (per reference):
  B=16, N=1024, F=64, T=12, K=3, F_OUT=64
  S = softmax_axis1(Vs @ sigmoid(lhs @ rhs^T + bs))
  out = relu(sum_k (cheb_k * S)^T @ x @ Theta_k)

Sharding: batch B=16 split across 8 cores (2 batches/core). All weights
replicated. Each core runs an identical Bass program on its shard.

v2 design notes:
  - All large matmuls use 512-wide moving operands (one PSUM bank), with
    the attention path (Vs/P/E/cheb/A/x') in bf16 (errors ~0.5%, well
    under the 2e-2 gate) so the hot set fits in SBUF and FWL kicks in.
  - Per batch: stage A (x load/reorder + small reductions), stage P
    (product + bs -> sigmoid, sigmoids grouped to avoid act-table
    thrash), then per j-half: S-accumulate -> exp -> colsum, A_k =
    cheb_k * E (split across DVE and Pool engines), z via x'-stationary
    matmuls, Theta via block-diag stationary accumulated across k in
    PSUM, then PE transpose + fused relu*recip writeback.
  - Softmax denominator folded into the final relu as a per-partition
    scale (partition = destination node j after the transpose).
"""
import numpy as np

import concourse.bass as bass
import concourse.mybir as mybir
import concourse.tile as tile
from concourse import bacc
from concourse.masks import make_identity

F32 = mybir.dt.float32
F32R = mybir.dt.float32r
BF16 = mybir.dt.bfloat16
AL = mybir.AluOpType
AF = mybir.ActivationFunctionType
AX = mybir.AxisListType

B_PER_CORE = 2
N = 1024
F = 64
T = 12
K = 3
FO = 64
NT = N // 128           # 8 node tiles
JH = 512                # j processed in halves of 512
NJH = N // JH           # 2
TF = (T * F) // 128     # 6 (t,f)-chunks (each = 2 t-values x 64 f)


def _emit_front(nc, pools, cst, b, x_d):
    """Stage A (x load + row features) + stage P (product+bs -> sigmoid) +
    the x' reorder. Returns the state the j-half sections consume."""
    (sbp, psMain, psOut, dram_pool) = pools
    ident = cst["ident"]

    xw1T = sbp.tile([F, N], F32R, tag="xw1T", bufs=1, name="xw1T")
    rhsBT = sbp.tile([T, N], F32R, tag="rhsBT", bufs=1, name="rhsBT")
    xnats = []
    for i in range(NT):
        xnat = sbp.tile([128, F, T], F32, tag="xnat", bufs=4, name="xnat")
        nc.sync.dma_start(out=xnat[:], in_=x_d.ap()[b, i * 128:(i + 1) * 128])
        xnats.append(xnat)
        # xw1[n,f] = sum_t x*W1   (mult on Pool, reduce on DVE)
        tmpA = sbp.tile([128, F, T], BF16, tag="tmpA", bufs=1, name="tmpA")
        nc.gpsimd.tensor_mul(tmpA[:], xnat[:], cst["w1rep"][:])
        xw1_i = sbp.tile([128, F], F32, tag="xw1i", bufs=2, name="xw1_i")
        nc.vector.tensor_reduce(out=xw1_i[:], in_=tmpA[:], op=AL.add, axis=AX.X)
        # rhsB[n,t] = sum_f W3*x   (mult on Pool, reduce on DVE)
        tmpB = sbp.tile([128, T, F], BF16, tag="tmpB", bufs=1, name="tmpB")
        nc.gpsimd.tensor_mul(tmpB[:], xnat[:].rearrange("p f t -> p t f"),
                             cst["w3rep"][:])
        rhsB_i = sbp.tile([128, T], F32, tag="rhsBi", bufs=2, name="rhsB_i")
        nc.vector.tensor_reduce(out=rhsB_i[:], in_=tmpB[:], op=AL.add, axis=AX.X)
        # transpose both to contraction-on-partitions layout
        ps_t1 = psMain.tile([F, 128], F32, tag="m", name="ps_t1")
        nc.tensor.transpose(ps_t1[:], xw1_i[:], ident[:])
        nc.scalar.copy(xw1T[:, i * 128:(i + 1) * 128], ps_t1[:])
        ps_t2 = psMain.tile([T, 128], F32, tag="m", name="ps_t2")
        nc.tensor.transpose(ps_t2[:], rhsB_i[:], ident[:])
        nc.scalar.copy(rhsBT[:, i * 128:(i + 1) * 128], ps_t2[:])

    # lhsT[t, u] = sum_f W2[f,t] * xw1T[f, u]
    lhsT_sb = sbp.tile([T, N], F32R, tag="lhsT", bufs=1, name="lhsT_sb")
    for h in range(2):
        ps_l = psMain.tile([T, JH], F32, tag="m", name="ps_l")
        nc.tensor.matmul(ps_l[:], cst["w2r"][:], xw1T[:, h * JH:(h + 1) * JH],
                         start=True, stop=True)
        nc.scalar.copy(lhsT_sb[:, h * JH:(h + 1) * JH], ps_l[:])

    # Stage P: product + bs -> sigmoid. bs is accumulated into PSUM via an
    # identity-stationary matmul so the sigmoid reads PSUM directly.
    P_sb = []
    for jh in range(NJH):
        JS = slice(jh * JH, (jh + 1) * JH)
        P_h = sbp.tile([128, NT, JH], BF16, tag="P", bufs=3, name="P_h")
        for u in range(NT):
            ps_p = psMain.tile([128, JH], F32, tag="m", name="ps_p")
            nc.tensor.matmul(ps_p[:], lhsT_sb[:, u * 128:(u + 1) * 128],
                             rhsBT[:, JS], start=True, stop=False)
            nc.tensor.matmul(ps_p[:], cst["ident_bf"][:], cst["bs"][:, u, JS],
                             start=False, stop=True)
            nc.scalar.activation(P_h[:, u], ps_p[:], AF.Sigmoid)
        P_sb.append(P_h)

    # x' reorder (f,t) -> (t,f), bf16 (act engine; stationary of z matmuls)
    xprime = sbp.tile([128, NT, T, F], BF16, tag="xp", bufs=2, name="xprime")
    for i in range(NT):
        nc.scalar.copy(xprime[:, i], xnats[i][:].rearrange("p f t -> p t f"))

    return {"P": P_sb, "xprime": xprime}


def _emit_jh(nc, pools, cst, st, b, jh, cheb_d, out_d, flush):
    """One j-half: S-accumulate -> exp -> colsum -> A_k -> z -> Theta.
    `flush` (the previous j-half's output drain) is emitted inside the S
    window so the PE never waits on PSUM drains. Returns this j-half's own
    output-drain closure."""
    (sbp, psMain, psOut, dram_pool) = pools
    identr = cst["identr"]
    xprime = st["xprime"]
    P_h = st["P"][jh]
    JS = slice(jh * JH, (jh + 1) * JH)

    # cheb prefetch for all 3 k of this j-half (SWDGE cast f32->bf16)
    cheb_t = []
    for k in range(K):
        ct = sbp.tile([128, NT, JH], BF16, tag="cheb", bufs=2, name="cheb_t")
        nc.gpsimd.dma_start(
            out=ct[:],
            in_=cheb_d.ap()[k, :, JS].rearrange("(i p) n -> p i n", p=128))
        cheb_t.append(ct)

    # S = Vs^T-stationary accumulation; E = exp(S)
    E_q = sbp.tile([128, NT, JH], BF16, tag="E", bufs=2, name="E_q")
    for i in range(NT):
        ps_s = psMain.tile([128, JH], F32, tag="m", name="ps_s")
        for u in range(NT):
            nc.tensor.matmul(ps_s[:], cst["vsT"][:, u, i * 128:(i + 1) * 128],
                             P_h[:, u], start=(u == 0), stop=(u == NT - 1))
        nc.scalar.activation(E_q[:, i], ps_s[:], AF.Exp)
        if i == 0 and flush is not None:
            flush()

    # colsum d[j] = sum_i E[i, j] via ones-stationary matmul
    ps_cs = psMain.tile([128, JH], F32, tag="m", name="ps_cs")
    for i in range(NT):
        nc.tensor.matmul(ps_cs[0:1, :], cst["ones_bf"][:], E_q[:, i],
                         start=(i == 0), stop=(i == NT - 1))
    cs_sb = sbp.tile([1, JH], F32, tag="cs", bufs=2, name="cs_sb")
    nc.scalar.copy(cs_sb[:], ps_cs[0:1, :])
    rc_sb = sbp.tile([1, JH], F32, tag="rc", bufs=2, name="rc_sb")
    nc.vector.reciprocal(rc_sb[:], cs_sb[:])
    rc_d = dram_pool.tile([JH], F32, tag="rcd", name="rc_d")
    nc.gpsimd.dma_start(out=rc_d.rearrange("(a b) -> a b", a=1), in_=rc_sb[:])
    recip_sb = sbp.tile([128, JH // 128], F32, tag="recip", bufs=2,
                        name="recip_sb")
    nc.gpsimd.dma_start(out=recip_sb[:],
                        in_=rc_d.rearrange("(c p) -> p c", p=128))

    # A_k = cheb_k * E, k-major (k=0 on DVE so the z matmuls' first operands
    # land earliest; k=1,2 on Pool)
    A_q = [sbp.tile([128, NT, JH], BF16, tag="A", bufs=3, name=f"A_q{k}")
           for k in range(K)]
    for k, eng in ((0, nc.vector), (1, nc.gpsimd), (2, nc.gpsimd)):
        for i in range(NT):
            eng.tensor_mul(A_q[k][:, i], cheb_t[k][:, i], E_q[:, i])

    # z' = x'-stationary matmuls; Theta via block-diag accumulated over k
    psOut_t = psOut.tile([128, TF, JH], F32, tag="out", name="psOut_t")
    theta_pending = []  # software-pipelined Theta matmuls
    for k in range(K):
        for tf in range(TF):
            ps_z = psMain.tile([128, JH], F32, tag="m", name="ps_z")
            for i in range(NT):
                nc.tensor.matmul(ps_z[:],
                                 xprime[:, i].rearrange("p t f -> p (t f)")
                                 [:, tf * 128:(tf + 1) * 128],
                                 A_q[k][:, i],
                                 start=(i == 0), stop=(i == NT - 1))
            if theta_pending:
                theta_pending.pop(0)()
            z_sb = sbp.tile([128, JH], F32R, tag="zsb", bufs=3, name="z_sb")
            if tf % 2 == 0:
                nc.scalar.copy(z_sb[:], ps_z[:])
            else:
                nc.vector.tensor_copy(z_sb[:], ps_z[:])

            def _mk(k=k, tf=tf, z_sb=z_sb, psOut_t=psOut_t):
                def _do():
                    nc.tensor.matmul(psOut_t[:, tf], cst["thbd"][:, k, :],
                                     z_sb[:], start=(k == 0), stop=(k == K - 1))
                return _do
            theta_pending.append(_mk())
    for fn in theta_pending:
        fn()

    def _flush_out():
        # drain psOut -> SBUF, transpose, fused relu * recip writeback
        oT = sbp.tile([128, TF, JH], F32R, tag="oT", bufs=1, name="oT")
        for tf in range(TF):
            if tf % 2 == 0:
                nc.scalar.copy(oT[:, tf], psOut_t[:, tf])
            else:
                nc.vector.tensor_copy(oT[:, tf], psOut_t[:, tf])
        for js in range(JH // 128):
            res = sbp.tile([128, FO, T], F32, tag="res", bufs=2, name="res")
            for g in range(2):
                ps_tr = psMain.tile([128, 384], F32R, tag="m", name="ps_tr")
                for q in range(3):
                    nc.tensor.transpose(
                        ps_tr[:, q * 128:(q + 1) * 128],
                        oT[:, g * 3 + q, js * 128:(js + 1) * 128], identr[:])
                nc.scalar.activation(
                    res[:].rearrange("p o (gg q dt) -> p gg q dt o", gg=2, q=3,
                                     dt=2)[:, g],
                    ps_tr[:].rearrange("p (q dt o) -> p q dt o", q=3, o=FO),
                    AF.Relu, scale=recip_sb[:, js:js + 1])
            nj = jh * (JH // 128) + js
            nc.sync.dma_start(out=out_d.ap()[b, nj * 128:(nj + 1) * 128],
                              in_=res[:])

    return _flush_out


def build_nc(repeat=1):
    nc = bacc.Bacc("TRN2", target_bir_lowering=False, debug=False, num_devices=8)
    x_d = nc.dram_tensor("x", [B_PER_CORE, N, F, T], F32, kind="ExternalInput")
    w1_d = nc.dram_tensor("W1", [T], F32, kind="ExternalInput")
    w2_d = nc.dram_tensor("W2", [F, T], F32, kind="ExternalInput")
    w3_d = nc.dram_tensor("W3", [F], F32, kind="ExternalInput")
    bs_d = nc.dram_tensor("bs", [N, N], F32, kind="ExternalInput")
    vs_d = nc.dram_tensor("Vs", [N, N], F32, kind="ExternalInput")
    cheb_d = nc.dram_tensor("cheb", [K, N, N], F32, kind="ExternalInput")
    th_d = nc.dram_tensor("Theta", [K, F, FO], F32, kind="ExternalInput")
    out_d = nc.dram_tensor("out", [B_PER_CORE, N, FO, T], F32,
                           kind="ExternalOutput")

    with tile.TileContext(nc) as tc:
        with (
            tc.tile_pool(name="consts", bufs=1) as consts,
            tc.tile_pool(name="sbp", bufs=1) as sbp,
            tc.tile_pool(name="dram", bufs=2, space="DRAM") as dram_pool,
            tc.tile_pool(name="psMain", bufs=2, space="PSUM") as psMain,
            tc.tile_pool(name="psOut", bufs=1, space="PSUM") as psOut,
        ):
            cst = {}
            ident = consts.tile([128, 128], F32)
            make_identity(nc, ident[:])
            cst["ident"] = ident
            identr = consts.tile([128, 128], F32R)
            nc.vector.tensor_copy(identr[:], ident[:])
            cst["identr"] = identr
            onesf = consts.tile([128, 1], F32)
            nc.vector.memset(onesf[:], 1.0)
            ones_bf = consts.tile([128, 1], BF16)
            nc.vector.tensor_copy(ones_bf[:], onesf[:])
            cst["ones_bf"] = ones_bf
            ident_bf = consts.tile([128, 128], BF16)
            nc.vector.tensor_copy(ident_bf[:], ident[:])
            cst["ident_bf"] = ident_bf
            # broadcast W1 / W3 replicas
            w1rep = consts.tile([128, F, T], F32)
            nc.gpsimd.dma_start(
                out=w1rep[:],
                in_=bass.AP(tensor=w1_d, offset=0, ap=[[0, 128], [0, F], [1, T]]))
            cst["w1rep"] = w1rep
            w3rep = consts.tile([128, T, F], F32)
            nc.gpsimd.dma_start(
                out=w3rep[:],
                in_=bass.AP(tensor=w3_d, offset=0, ap=[[0, 128], [0, T], [1, F]]))
            cst["w3rep"] = w3rep
            # W2 (f, t) fp32r
            w2f = consts.tile([F, T], F32)
            nc.sync.dma_start(out=w2f[:], in_=w2_d.ap())
            w2r = consts.tile([F, T], F32R)
            nc.vector.tensor_copy(w2r[:], w2f[:])
            cst["w2r"] = w2r
            # bs resident, bf16 (cast during SWDGE DMA)
            bs_sb = consts.tile([128, NT, N], BF16, name="bs_sb")
            nc.gpsimd.dma_start(
                out=bs_sb[:],
                in_=bs_d.ap().rearrange("(u p) n -> p u n", p=128))
            cst["bs"] = bs_sb
            # block-diagonal Theta (128, K, 128) fp32r
            thbd_f = consts.tile([128, K, 128], F32)
            nc.vector.memset(thbd_f[:], 0.0)
            for k in range(K):
                nc.sync.dma_start(out=thbd_f[0:F, k, 0:FO], in_=th_d.ap()[k])
                nc.sync.dma_start(out=thbd_f[F:128, k, FO:128], in_=th_d.ap()[k])
            thbd = consts.tile([128, K, 128], F32R)
            nc.vector.tensor_copy(thbd[:], thbd_f[:])
            cst["thbd"] = thbd
            # VsT (u-partitioned Vs transpose), bf16
            vsT = consts.tile([128, NT, N], BF16, name="vsT")
            for ut in range(NT):
                for it in range(NT):
                    vtmp = sbp.tile([128, 128], F32, tag="vtmp", bufs=2,
                                    name="vtmp")
                    nc.sync.dma_start(
                        out=vtmp[:],
                        in_=vs_d.ap()[it * 128:(it + 1) * 128,
                                      ut * 128:(ut + 1) * 128])
                    ps_v = psMain.tile([128, 128], F32, tag="m", name="ps_v")
                    nc.tensor.transpose(ps_v[:], vtmp[:], ident[:])
                    nc.scalar.copy(vsT[:, ut, it * 128:(it + 1) * 128], ps_v[:])
            cst["vsT"] = vsT

            pools = (sbp, psMain, psOut, dram_pool)
            # Software pipeline across batches: the next batch's front
            # (stage A + P) is emitted between the two j-half sections of
            # the current batch, and each j-half's output drain is flushed
            # inside the next S window.
            seq = [b for _ in range(repeat) for b in range(B_PER_CORE)]
            states = [_emit_front(nc, pools, cst, seq[0], x_d)]
            flush = None
            for idx, b in enumerate(seq):
                st = states[idx]
                flush = _emit_jh(nc, pools, cst, st, b, 0, cheb_d, out_d, flush)
                if idx + 1 < len(seq):
                    states.append(_emit_front(nc, pools, cst, seq[idx + 1], x_d))
                flush = _emit_jh(nc, pools, cst, st, b, 1, cheb_d, out_d, flush)
            flush()
    nc.compile()
    return nc


_RUNNER_CACHE = {}


def _make_runner(repeat=1):
    """Build the Bass program once and wrap it in a persistent jitted
    shard_map executable so repeat calls skip recompile/reload."""
    import jax
    from jax.sharding import Mesh, PartitionSpec
    from jax.experimental.shard_map import shard_map
    from concourse import bass2jax, mybir as _mybir

    nc = build_nc(repeat)
    bass2jax.install_neuronx_cc_hook()

    part_name = nc.partition_id_tensor.name if nc.partition_id_tensor else None
    in_names = []
    out_names = []
    out_avals = []
    zero_outs = []
    for alloc in nc.m.functions[0].allocations:
        if not isinstance(_mybir.MemoryLocationSet, type) or not isinstance(
                alloc, _mybir.MemoryLocationSet):
            continue
        name = alloc.memorylocations[0].name
        if alloc.kind == "ExternalInput":
            if name != part_name:
                in_names.append(name)
        elif alloc.kind == "ExternalOutput":
            out_names.append(name)
            shape = tuple(alloc.tensor_shape)
            dtype = _mybir.dt.np(alloc.dtype)
            out_avals.append(jax.core.ShapedArray(shape, dtype))
            zero_outs.append(np.zeros(shape, dtype))
    n_params = len(in_names)
    all_names = in_names + out_names
    if part_name is not None:
        all_names = all_names + [part_name]

    def _body(*args):
        operands = list(args)
        if part_name is not None:
            operands.append(bass2jax.partition_id_tensor())
        outs = bass2jax._bass_exec_p.bind(
            *operands,
            out_avals=tuple(out_avals),
            in_names=tuple(all_names),
            out_names=tuple(out_names),
            lowering_input_output_aliases=(),
            sim_require_finite=False,
            sim_require_nnan=False,
            nc=nc,
        )
        return tuple(outs)

    n_cores = 8
    devices = jax.devices()[:n_cores]
    mesh = Mesh(np.asarray(devices), ("core",))
    in_specs = tuple(
        PartitionSpec("core") if name == "x" else PartitionSpec()
        for name in in_names
    ) + (PartitionSpec("core"),) * len(out_names)
    out_specs = (PartitionSpec("core"),) * len(out_names)
    sharded = jax.jit(
        shard_map(_body, mesh=mesh, in_specs=in_specs, out_specs=out_specs,
                  check_rep=False),
        keep_unused=True,
    )
    return nc, sharded, in_names, out_names, zero_outs, n_cores, mesh


def _get_runner(repeat=1):
    if repeat not in _RUNNER_CACHE:
        _RUNNER_CACHE[repeat] = _make_runner(repeat)
    return _RUNNER_CACHE[repeat]


def kernel(x, W1, W2, W3, bs, Vs, cheb, Theta, repeat=1):
    x = np.asarray(x, dtype=np.float32)
    full = {
        "W1": np.asarray(W1, dtype=np.float32),
        "W2": np.asarray(W2, dtype=np.float32),
        "W3": np.asarray(W3, dtype=np.float32),
        "bs": np.asarray(bs, dtype=np.float32).reshape(N, N),
        "Vs": np.asarray(Vs, dtype=np.float32),
        "cheb": np.asarray(cheb, dtype=np.float32),
        "Theta": np.asarray(Theta, dtype=np.float32),
    }
    nc, sharded, in_names, out_names, zero_outs, n_cores, mesh = _get_runner(repeat)
    ops = _staged_ops(x, full, in_names, zero_outs, n_cores)
    out_arrs = sharded(*ops)
    out = np.asarray(out_arrs[out_names.index("out")])
    return out.reshape(16, N, FO, T)


def _staged_ops(x, full, in_names, zero_outs, n_cores):
    ops = []
    for name in in_names:
        if name == "x":
            ops.append(np.ascontiguousarray(x.reshape(n_cores * B_PER_CORE, N, F, T)))
        else:
            ops.append(full[name])
    for z in zero_outs:
        ops.append(np.zeros((n_cores * z.shape[0], *z.shape[1:]), z.dtype))
    return ops


def _bench_setup(inputs, repeat):
    import jax
    from jax.sharding import NamedSharding, PartitionSpec
    x = np.asarray(inputs["x"], dtype=np.float32)
    full = {k: np.asarray(v, dtype=np.float32) for k, v in inputs.items() if k != "x"}
    full["bs"] = full["bs"].reshape(N, N)
    nc, sharded, in_names, out_names, zero_outs, n_cores, mesh = _get_runner(repeat)
    ops = _staged_ops(x, full, in_names, zero_outs, n_cores)
    sh_core = NamedSharding(mesh, PartitionSpec("core"))
    sh_rep = NamedSharding(mesh, PartitionSpec())
    shardings = [sh_core if name == "x" else sh_rep for name in in_names]
    shardings += [sh_core] * len(zero_outs)
    dev_ops = [jax.device_put(o, s_) for o, s_ in zip(ops, shardings)]
    jax.block_until_ready(sharded(*dev_ops))
    return sharded, dev_ops


def bench_pair(inputs, rep_a=1, rep_b=9, iters=20):
    """Interleaved device-resident timing of two repeat variants.
    Returns (best_a, best_b) seconds — interleaving cancels slow drift in the
    fixed dispatch overhead."""
    import time as _time
    import jax
    sh_a, ops_a = _bench_setup(inputs, rep_a)
    sh_b, ops_b = _bench_setup(inputs, rep_b)
    best_a = best_b = float("inf")
    for _ in range(iters):
        t0 = _time.time()
        jax.block_until_ready(sh_a(*ops_a))
        best_a = min(best_a, _time.time() - t0)
        t0 = _time.time()
        jax.block_until_ready(sh_b(*ops_b))
        best_b = min(best_b, _time.time() - t0)
    return best_a, best_b


# revision 16
# speedup vs baseline: 16.2480x; 16.2480x over previous
"""MAMGCN submodule kernel for Trainium2, 8-core data-parallel over batch.

Problem (per reference):
  B=16, N=1024, F=64, T=12, K=3, F_OUT=64
  S = softmax_axis1(Vs @ sigmoid(lhs @ rhs^T + bs))
  out = relu(sum_k (cheb_k * S)^T @ x @ Theta_k)

Sharding: batch B=16 split across 8 cores (2 batches/core). All weights
replicated. Each core runs an identical Bass program on its shard.

v2 design notes:
  - All large matmuls use 512-wide moving operands (one PSUM bank), with
    the attention path (Vs/P/E/cheb/A/x') in bf16 (errors ~0.5%, well
    under the 2e-2 gate) so the hot set fits in SBUF and FWL kicks in.
  - Per batch: stage A (x load/reorder + small reductions), stage P
    (product + bs -> sigmoid, sigmoids grouped to avoid act-table
    thrash), then per j-half: S-accumulate -> exp -> colsum, A_k =
    cheb_k * E (split across DVE and Pool engines), z via x'-stationary
    matmuls, Theta via block-diag stationary accumulated across k in
    PSUM, then PE transpose + fused relu*recip writeback.
  - Softmax denominator folded into the final relu as a per-partition
    scale (partition = destination node j after the transpose).
"""
import numpy as np

import concourse.bass as bass
import concourse.mybir as mybir
import concourse.tile as tile
from concourse import bacc
from concourse.masks import make_identity

F32 = mybir.dt.float32
F32R = mybir.dt.float32r
BF16 = mybir.dt.bfloat16
AL = mybir.AluOpType
AF = mybir.ActivationFunctionType
AX = mybir.AxisListType

B_PER_CORE = 2
N = 1024
F = 64
T = 12
K = 3
FO = 64
NT = N // 128           # 8 node tiles
JH = 512                # j processed in halves of 512
NJH = N // JH           # 2
TF = (T * F) // 128     # 6 (t,f)-chunks (each = 2 t-values x 64 f)


def _plan_front(nc, pools, cst, b, x_d):
    """Closures for one batch's front. `chunks[i]` does the per-i-tile DVE/
    Pool work (x load, W1/W3 reductions, x' reorder) — interleaved into the
    previous batch's z-phase so no engine FIFO head-blocks. `main()` emits
    the PE/act block (transposes, lhsT, product+bs, sigmoids)."""
    (sbp, psMain, psOut, dram_pool) = pools
    ident = cst["ident"]
    st = {}

    def _chunk(i):
        def _do():
            if i == 0:
                st["xnats"] = []
                st["xw1s"] = []
                st["rhsBs"] = []
                st["xprime"] = sbp.tile([128, NT, T, F], BF16, tag="xp",
                                        bufs=2, name="xprime")
            xnat = sbp.tile([128, F, T], F32, tag="xnat", bufs=4, name="xnat")
            nc.sync.dma_start(out=xnat[:],
                              in_=x_d.ap()[b, i * 128:(i + 1) * 128])
            st["xnats"].append(xnat)
            # xw1[n,f] = sum_t x*W1   (mult on Pool, reduce on DVE)
            tmpA = sbp.tile([128, F, T], BF16, tag="tmpA", bufs=2, name="tmpA")
            nc.gpsimd.tensor_mul(tmpA[:], xnat[:], cst["w1rep"][:])
            xw1_i = sbp.tile([128, F], F32, tag="xw1i", bufs=NT, name="xw1_i")
            nc.vector.tensor_reduce(out=xw1_i[:], in_=tmpA[:], op=AL.add,
                                    axis=AX.X)
            # rhsB[n,t] = sum_f W3*x   (DVE, strided read)
            tmpB = sbp.tile([128, T, F], BF16, tag="tmpB", bufs=2, name="tmpB")
            nc.vector.tensor_mul(tmpB[:], xnat[:].rearrange("p f t -> p t f"),
                                 cst["w3rep"][:])
            rhsB_i = sbp.tile([128, T], F32, tag="rhsBi", bufs=NT,
                              name="rhsB_i")
            nc.vector.tensor_reduce(out=rhsB_i[:], in_=tmpB[:], op=AL.add,
                                    axis=AX.X)
            # x' reorder (f,t) -> (t,f), bf16 (frees xnat early)
            nc.vector.tensor_copy(st["xprime"][:, i],
                                  xnat[:].rearrange("p f t -> p t f"))
            st["xw1s"].append(xw1_i)
            st["rhsBs"].append(rhsB_i)
        return _do

    def main():
        xw1T = sbp.tile([F, N], F32R, tag="xw1T", bufs=1, name="xw1T")
        rhsBT = sbp.tile([T, N], F32R, tag="rhsBT", bufs=1, name="rhsBT")
        for i in range(NT):
            ps_t1 = psMain.tile([F, 128], F32, tag="m", name="ps_t1")
            nc.tensor.transpose(ps_t1[:], st["xw1s"][i][:], ident[:])
            nc.scalar.copy(xw1T[:, i * 128:(i + 1) * 128], ps_t1[:])
            ps_t2 = psMain.tile([T, 128], F32, tag="m", name="ps_t2")
            nc.tensor.transpose(ps_t2[:], st["rhsBs"][i][:], ident[:])
            nc.scalar.copy(rhsBT[:, i * 128:(i + 1) * 128], ps_t2[:])

        # lhsT[t, u] = sum_f W2[f,t] * xw1T[f, u]
        lhsT_sb = sbp.tile([T, N], F32R, tag="lhsT", bufs=1, name="lhsT_sb")
        for h in range(2):
            ps_l = psMain.tile([T, JH], F32, tag="m", name="ps_l")
            nc.tensor.matmul(ps_l[:], cst["w2r"][:],
                             xw1T[:, h * JH:(h + 1) * JH],
                             start=True, stop=True)
            nc.scalar.copy(lhsT_sb[:, h * JH:(h + 1) * JH], ps_l[:])

        # product + bs -> sigmoid; bs accumulated in PSUM via an
        # identity-stationary matmul (no DVE hop before the sigmoid)
        P_sb = []
        for jh in range(NJH):
            JS = slice(jh * JH, (jh + 1) * JH)
            P_h = sbp.tile([128, NT, JH], BF16, tag="P", bufs=3, name="P_h")
            for u in range(NT):
                ps_p = psMain.tile([128, JH], F32, tag="m", name="ps_p")
                nc.tensor.matmul(ps_p[:], lhsT_sb[:, u * 128:(u + 1) * 128],
                                 rhsBT[:, JS], start=True, stop=False)
                nc.tensor.matmul(ps_p[:], cst["ident_bf"][:],
                                 cst["bs"][:, u, JS], start=False, stop=True)
                nc.scalar.activation(P_h[:, u], ps_p[:], AF.Sigmoid)
            P_sb.append(P_h)
        st["P"] = P_sb

    st["chunks"] = [_chunk(i) for i in range(NT)]
    st["main"] = main
    return st


def _emit_jh(nc, pools, cst, st, b, jh, cheb_d, out_d, flush_a, flush_b,
             fillers):
    """One j-half: S-accumulate -> exp -> colsum -> A_k -> z -> Theta.
    `flush_a` (previous section's PSUM-out drain) and `flush_b` (its
    transpose + relu*recip writeback) are emitted inside this S window;
    `fillers` (next batch's front chunks) are sprinkled into the z-loop.
    Returns this section's own (flush_a, flush_b)."""
    (sbp, psMain, psOut, dram_pool) = pools
    identr = cst["identr"]
    xprime = st["xprime"]
    P_h = st["P"][jh]
    JS = slice(jh * JH, (jh + 1) * JH)

    # cheb prefetch for all 3 k of this j-half (SWDGE cast f32->bf16)
    cheb_t = []
    for k in range(K):
        ct = sbp.tile([128, NT, JH], BF16, tag="cheb", bufs=2, name="cheb_t")
        nc.gpsimd.dma_start(
            out=ct[:],
            in_=cheb_d.ap()[k, :, JS].rearrange("(i p) n -> p i n", p=128))
        cheb_t.append(ct)

    # S = Vs^T-stationary accumulation; E = exp(S)
    E_q = sbp.tile([128, NT, JH], BF16, tag="E", bufs=1, name="E_q")
    for i in range(NT):
        ps_s = psMain.tile([128, JH], F32, tag="m", name="ps_s")
        for u in range(NT):
            nc.tensor.matmul(ps_s[:], cst["vsT"][:, u, i * 128:(i + 1) * 128],
                             P_h[:, u], start=(u == 0), stop=(u == NT - 1))
        nc.scalar.activation(E_q[:, i], ps_s[:], AF.Exp)
        if i == 0 and flush_a is not None:
            flush_a()
        if i == 2 and flush_b is not None:
            flush_b()

    # colsum d[j] = sum_i E[i, j] via ones-stationary matmul
    ps_cs = psMain.tile([128, JH], F32, tag="m", name="ps_cs")
    for i in range(NT):
        nc.tensor.matmul(ps_cs[0:1, :], cst["ones_bf"][:], E_q[:, i],
                         start=(i == 0), stop=(i == NT - 1))
    cs_sb = sbp.tile([1, JH], F32, tag="cs", bufs=2, name="cs_sb")
    nc.scalar.copy(cs_sb[:], ps_cs[0:1, :])
    rc_sb = sbp.tile([1, JH], F32, tag="rc", bufs=2, name="rc_sb")
    nc.vector.reciprocal(rc_sb[:], cs_sb[:])

    # A_k = cheb_k * E, k-major (k=0 on DVE, k=1,2 on Pool)
    A_q = [sbp.tile([128, NT, JH], BF16, tag="A", bufs=3, name=f"A_q{k}")
           for k in range(K)]
    for k, eng in ((0, nc.vector), (1, nc.gpsimd), (2, nc.gpsimd)):
        for i in range(NT):
            eng.tensor_mul(A_q[k][:, i], cheb_t[k][:, i], E_q[:, i])

    # softmax denominators -> per-partition layout (needed only by flush_b)
    rc_d = dram_pool.tile([JH], F32, tag="rcd", name="rc_d")
    nc.gpsimd.dma_start(out=rc_d.rearrange("(a b) -> a b", a=1), in_=rc_sb[:])
    recip_sb = sbp.tile([128, JH // 128], F32, tag="recip", bufs=2,
                        name="recip_sb")
    nc.gpsimd.dma_start(out=recip_sb[:],
                        in_=rc_d.rearrange("(c p) -> p c", p=128))

    # z' = x'-stationary matmuls; Theta via block-diag accumulated over k
    psOut_t = psOut.tile([128, TF, JH], F32, tag="out", name="psOut_t")
    theta_pending = []
    fillers = list(fillers)
    for k in range(K):
        for tf in range(TF):
            ps_z = psMain.tile([128, JH], F32, tag="m", name="ps_z")
            for i in range(NT):
                nc.tensor.matmul(ps_z[:],
                                 xprime[:, i].rearrange("p t f -> p (t f)")
                                 [:, tf * 128:(tf + 1) * 128],
                                 A_q[k][:, i],
                                 start=(i == 0), stop=(i == NT - 1))
            if theta_pending:
                theta_pending.pop(0)()
            z_sb = sbp.tile([128, JH], F32R, tag="zsb", bufs=3, name="z_sb")
            nc.scalar.copy(z_sb[:], ps_z[:])
            if (k * TF + tf) % 2 == 0 and fillers:
                fillers.pop(0)()

            def _mk(k=k, tf=tf, z_sb=z_sb, psOut_t=psOut_t):
                def _do():
                    nc.tensor.matmul(psOut_t[:, tf], cst["thbd"][:, k, :],
                                     z_sb[:], start=(k == 0),
                                     stop=(k == K - 1))
                return _do
            theta_pending.append(_mk())
    for fn in theta_pending:
        fn()
    for fn in fillers:
        fn()

    def _flush_a():
        # drain psOut -> SBUF (split act/DVE)
        st["oT"] = oT = sbp.tile([128, TF, JH], F32R, tag="oT", bufs=1,
                                 name="oT")
        for tf in range(TF):
            if tf % 2 == 0:
                nc.scalar.copy(oT[:, tf], psOut_t[:, tf])
            else:
                nc.vector.tensor_copy(oT[:, tf], psOut_t[:, tf])

    def _flush_b():
        # transpose + fused relu * recip writeback + store
        oT = st["oT"]
        for js in range(JH // 128):
            res = sbp.tile([128, FO, T], F32, tag="res", bufs=2, name="res")
            for g in range(2):
                ps_tr = psMain.tile([128, 384], F32R, tag="m", name="ps_tr")
                for q in range(3):
                    nc.tensor.transpose(
                        ps_tr[:, q * 128:(q + 1) * 128],
                        oT[:, g * 3 + q, js * 128:(js + 1) * 128], identr[:])
                nc.scalar.activation(
                    res[:].rearrange("p o (gg q dt) -> p gg q dt o", gg=2,
                                     q=3, dt=2)[:, g],
                    ps_tr[:].rearrange("p (q dt o) -> p q dt o", q=3, o=FO),
                    AF.Relu, scale=recip_sb[:, js:js + 1])
            nj = jh * (JH // 128) + js
            nc.sync.dma_start(out=out_d.ap()[b, nj * 128:(nj + 1) * 128],
                              in_=res[:])

    return _flush_a, _flush_b


def build_nc(repeat=1):
    nc = bacc.Bacc("TRN2", target_bir_lowering=False, debug=False, num_devices=8)
    x_d = nc.dram_tensor("x", [B_PER_CORE, N, F, T], F32, kind="ExternalInput")
    w1_d = nc.dram_tensor("W1", [T], F32, kind="ExternalInput")
    w2_d = nc.dram_tensor("W2", [F, T], F32, kind="ExternalInput")
    w3_d = nc.dram_tensor("W3", [F], F32, kind="ExternalInput")
    bs_d = nc.dram_tensor("bs", [N, N], F32, kind="ExternalInput")
    vs_d = nc.dram_tensor("Vs", [N, N], F32, kind="ExternalInput")
    cheb_d = nc.dram_tensor("cheb", [K, N, N], F32, kind="ExternalInput")
    th_d = nc.dram_tensor("Theta", [K, F, FO], F32, kind="ExternalInput")
    out_d = nc.dram_tensor("out", [B_PER_CORE, N, FO, T], F32,
                           kind="ExternalOutput")

    with tile.TileContext(nc) as tc:
        with (
            tc.tile_pool(name="consts", bufs=1) as consts,
            tc.tile_pool(name="sbp", bufs=1) as sbp,
            tc.tile_pool(name="dram", bufs=2, space="DRAM") as dram_pool,
            tc.tile_pool(name="psMain", bufs=2, space="PSUM") as psMain,
            tc.tile_pool(name="psOut", bufs=1, space="PSUM") as psOut,
        ):
            cst = {}
            ident = consts.tile([128, 128], F32)
            make_identity(nc, ident[:])
            cst["ident"] = ident
            identr = consts.tile([128, 128], F32R)
            nc.vector.tensor_copy(identr[:], ident[:])
            cst["identr"] = identr
            onesf = consts.tile([128, 1], F32)
            nc.vector.memset(onesf[:], 1.0)
            ones_bf = consts.tile([128, 1], BF16)
            nc.vector.tensor_copy(ones_bf[:], onesf[:])
            cst["ones_bf"] = ones_bf
            ident_bf = consts.tile([128, 128], BF16)
            nc.vector.tensor_copy(ident_bf[:], ident[:])
            cst["ident_bf"] = ident_bf
            # broadcast W1 / W3 replicas
            w1rep = consts.tile([128, F, T], F32)
            nc.gpsimd.dma_start(
                out=w1rep[:],
                in_=bass.AP(tensor=w1_d, offset=0, ap=[[0, 128], [0, F], [1, T]]))
            cst["w1rep"] = w1rep
            w3rep = consts.tile([128, T, F], F32)
            nc.gpsimd.dma_start(
                out=w3rep[:],
                in_=bass.AP(tensor=w3_d, offset=0, ap=[[0, 128], [0, T], [1, F]]))
            cst["w3rep"] = w3rep
            # W2 (f, t) fp32r
            w2f = consts.tile([F, T], F32)
            nc.sync.dma_start(out=w2f[:], in_=w2_d.ap())
            w2r = consts.tile([F, T], F32R)
            nc.vector.tensor_copy(w2r[:], w2f[:])
            cst["w2r"] = w2r
            # bs resident, bf16 (cast during SWDGE DMA)
            bs_sb = consts.tile([128, NT, N], BF16, name="bs_sb")
            nc.gpsimd.dma_start(
                out=bs_sb[:],
                in_=bs_d.ap().rearrange("(u p) n -> p u n", p=128))
            cst["bs"] = bs_sb
            # block-diagonal Theta (128, K, 128) fp32r
            thbd_f = consts.tile([128, K, 128], F32)
            nc.vector.memset(thbd_f[:], 0.0)
            for k in range(K):
                nc.sync.dma_start(out=thbd_f[0:F, k, 0:FO], in_=th_d.ap()[k])
                nc.sync.dma_start(out=thbd_f[F:128, k, FO:128], in_=th_d.ap()[k])
            thbd = consts.tile([128, K, 128], F32R)
            nc.vector.tensor_copy(thbd[:], thbd_f[:])
            cst["thbd"] = thbd
            # VsT (u-partitioned Vs transpose), bf16
            vsT = consts.tile([128, NT, N], BF16, name="vsT")
            for ut in range(NT):
                for it in range(NT):
                    vtmp = sbp.tile([128, 128], F32, tag="vtmp", bufs=2,
                                    name="vtmp")
                    nc.sync.dma_start(
                        out=vtmp[:],
                        in_=vs_d.ap()[it * 128:(it + 1) * 128,
                                      ut * 128:(ut + 1) * 128])
                    ps_v = psMain.tile([128, 128], F32, tag="m", name="ps_v")
                    nc.tensor.transpose(ps_v[:], vtmp[:], ident[:])
                    nc.scalar.copy(vsT[:, ut, it * 128:(it + 1) * 128], ps_v[:])
            cst["vsT"] = vsT

            pools = (sbp, psMain, psOut, dram_pool)
            # Software pipeline across batches: the next batch's front
            # chunks (DVE/Pool) are interleaved into the current batch's
            # jh0 z-phase, its PE/act block is emitted between the two
            # j-half sections, and each section's output drain is flushed
            # inside the next S window.
            seq = [b for _ in range(repeat) for b in range(B_PER_CORE)]
            plans = [_plan_front(nc, pools, cst, seq[0], x_d)]
            for c in plans[0]["chunks"]:
                c()
            plans[0]["main"]()
            fa = fb = None
            for idx, b in enumerate(seq):
                st = plans[idx]
                if idx + 1 < len(seq):
                    plans.append(_plan_front(nc, pools, cst, seq[idx + 1], x_d))
                    nxt = plans[idx + 1]["chunks"]
                else:
                    nxt = []
                fa, fb = _emit_jh(nc, pools, cst, st, b, 0, cheb_d, out_d,
                                  fa, fb, nxt)
                if idx + 1 < len(seq):
                    plans[idx + 1]["main"]()
                fa, fb = _emit_jh(nc, pools, cst, st, b, 1, cheb_d, out_d,
                                  fa, fb, [])
            fa()
            fb()
    nc.compile()
    return nc


_RUNNER_CACHE = {}


def _make_runner(repeat=1):
    """Build the Bass program once and wrap it in a persistent jitted
    shard_map executable so repeat calls skip recompile/reload."""
    import jax
    from jax.sharding import Mesh, PartitionSpec
    from jax.experimental.shard_map import shard_map
    from concourse import bass2jax, mybir as _mybir

    nc = build_nc(repeat)
    bass2jax.install_neuronx_cc_hook()

    part_name = nc.partition_id_tensor.name if nc.partition_id_tensor else None
    in_names = []
    out_names = []
    out_avals = []
    zero_outs = []
    for alloc in nc.m.functions[0].allocations:
        if not isinstance(_mybir.MemoryLocationSet, type) or not isinstance(
                alloc, _mybir.MemoryLocationSet):
            continue
        name = alloc.memorylocations[0].name
        if alloc.kind == "ExternalInput":
            if name != part_name:
                in_names.append(name)
        elif alloc.kind == "ExternalOutput":
            out_names.append(name)
            shape = tuple(alloc.tensor_shape)
            dtype = _mybir.dt.np(alloc.dtype)
            out_avals.append(jax.core.ShapedArray(shape, dtype))
            zero_outs.append(np.zeros(shape, dtype))
    n_params = len(in_names)
    all_names = in_names + out_names
    if part_name is not None:
        all_names = all_names + [part_name]

    def _body(*args):
        operands = list(args)
        if part_name is not None:
            operands.append(bass2jax.partition_id_tensor())
        outs = bass2jax._bass_exec_p.bind(
            *operands,
            out_avals=tuple(out_avals),
            in_names=tuple(all_names),
            out_names=tuple(out_names),
            lowering_input_output_aliases=(),
            sim_require_finite=False,
            sim_require_nnan=False,
            nc=nc,
        )
        return tuple(outs)

    n_cores = 8
    devices = jax.devices()[:n_cores]
    mesh = Mesh(np.asarray(devices), ("core",))
    in_specs = tuple(
        PartitionSpec("core") if name == "x" else PartitionSpec()
        for name in in_names
    ) + (PartitionSpec("core"),) * len(out_names)
    out_specs = (PartitionSpec("core"),) * len(out_names)
    sharded = jax.jit(
        shard_map(_body, mesh=mesh, in_specs=in_specs, out_specs=out_specs,
                  check_rep=False),
        keep_unused=True,
    )
    return nc, sharded, in_names, out_names, zero_outs, n_cores, mesh


def _get_runner(repeat=1):
    if repeat not in _RUNNER_CACHE:
        _RUNNER_CACHE[repeat] = _make_runner(repeat)
    return _RUNNER_CACHE[repeat]


def kernel(x, W1, W2, W3, bs, Vs, cheb, Theta, repeat=1):
    x = np.asarray(x, dtype=np.float32)
    full = {
        "W1": np.asarray(W1, dtype=np.float32),
        "W2": np.asarray(W2, dtype=np.float32),
        "W3": np.asarray(W3, dtype=np.float32),
        "bs": np.asarray(bs, dtype=np.float32).reshape(N, N),
        "Vs": np.asarray(Vs, dtype=np.float32),
        "cheb": np.asarray(cheb, dtype=np.float32),
        "Theta": np.asarray(Theta, dtype=np.float32),
    }
    nc, sharded, in_names, out_names, zero_outs, n_cores, mesh = _get_runner(repeat)
    ops = _staged_ops(x, full, in_names, zero_outs, n_cores)
    out_arrs = sharded(*ops)
    out = np.asarray(out_arrs[out_names.index("out")])
    return out.reshape(16, N, FO, T)


def _staged_ops(x, full, in_names, zero_outs, n_cores):
    ops = []
    for name in in_names:
        if name == "x":
            ops.append(np.ascontiguousarray(x.reshape(n_cores * B_PER_CORE, N, F, T)))
        else:
            ops.append(full[name])
    for z in zero_outs:
        ops.append(np.zeros((n_cores * z.shape[0], *z.shape[1:]), z.dtype))
    return ops


def _bench_setup(inputs, repeat):
    import jax
    from jax.sharding import NamedSharding, PartitionSpec
    x = np.asarray(inputs["x"], dtype=np.float32)
    full = {k: np.asarray(v, dtype=np.float32) for k, v in inputs.items() if k != "x"}
    full["bs"] = full["bs"].reshape(N, N)
    nc, sharded, in_names, out_names, zero_outs, n_cores, mesh = _get_runner(repeat)
    ops = _staged_ops(x, full, in_names, zero_outs, n_cores)
    sh_core = NamedSharding(mesh, PartitionSpec("core"))
    sh_rep = NamedSharding(mesh, PartitionSpec())
    shardings = [sh_core if name == "x" else sh_rep for name in in_names]
    shardings += [sh_core] * len(zero_outs)
    dev_ops = [jax.device_put(o, s_) for o, s_ in zip(ops, shardings)]
    jax.block_until_ready(sharded(*dev_ops))
    return sharded, dev_ops


def bench_pair(inputs, rep_a=1, rep_b=25, iters=30):
    """Interleaved device-resident timing of two repeat variants. Returns the
    median per-iteration marginal time in seconds — alternating the two
    programs and taking the median of adjacent-pair differences cancels both
    slow drift and bimodal jumps in the fixed dispatch overhead."""
    import time as _time
    import jax
    import numpy as _np
    sh_a, ops_a = _bench_setup(inputs, rep_a)
    sh_b, ops_b = _bench_setup(inputs, rep_b)
    diffs = []
    for _ in range(iters):
        t0 = _time.time()
        jax.block_until_ready(sh_a(*ops_a))
        ta = _time.time() - t0
        t0 = _time.time()
        jax.block_until_ready(sh_b(*ops_b))
        tb = _time.time() - t0
        diffs.append((tb - ta) / (rep_b - rep_a))
    return float(_np.median(_np.asarray(diffs)))
